# revision 15
# baseline (speedup 1.0000x reference)
"""Bass/Trainium2 kernel for nn_MaskedMHA (GQA decode attention + fused QKV projection).

Sharding: tensor-parallel over the 8 KV-head groups (1 kv head + its 4 q heads
per core).  Per-core device kernel, for its head group:
  qkv = x @ W_c^T + b_c          (bias folded in as an extra contraction k-tile)
  s[b,h,t] = q[b,h]·K[b,t] (+ mask*sqrt(D)), t < T past tokens only
  p = exp(s/sqrt(D))             (no max subtraction needed: |s| < ~8)
  av[b,h,:] = sum_{t<T} p[b,h,t]·V[b,t,:]       (unnormalized)
  s_new[(h,b), b'] = q[b,h]·k_new[b']           (one batched matmul; host uses diag)
outputs: av (batch-pair block layout), per-chunk softmax partial sums, s_new,
and (k_new, v_new).  Host epilogue (O(B*H*D)):
  pnew = exp(scale*s_new_diag + mask_T);  out = (av + pnew*v_new)/(sums+pnew);
  KV-cache concat (pure data movement).

Performance notes:
 - All big matmuls run as float32r (fp32 operands, fast PE mode: 1 cycle/row
   when the moving dim is >=256, vs 4 for plain fp32).
 - Batches are processed in PAIRS so the AV moving operand is [t, 256]
   (N=256 unlocks fast fp32r); the PSUM result holds two valid diagonal
   blocks, shipped whole, host slices them.
 - Scores PSUM = four rotating [4, 512] tiles (one per 512-chunk) so batch
   n+1's chunk-c matmul only waits exp(n, chunk c): cross-batch pipelining
   within the 8 PSUM banks.
 - DMA rings (each FIFO): sync=SP carries K pair-streams + av outs;
   gpsimd=SWDGE carries V pair-streams + mask rows; scalar=ACT carries
   xT/ident + small end outputs; wT chunks round-robin over all three.
"""
import numpy as np

B, PAST, NKV, NH, D, HID = 32, 2048, 8, 32, 128, 4096
GQ = NH // NKV          # 4 q-heads per kv group
SCALE = 1.0 / np.sqrt(np.float32(D))

_CACHE = {}
TRACE = False
LAST_RESULT = None


def build_nc(b, t, hid, with_mask=False):
    import concourse.tile as tile
    from concourse import bacc, mybir

    f32 = mybir.dt.float32
    f32r = mybir.dt.float32r
    od = GQ * D + 2 * D            # per-core fused qkv rows: 4*128 q + 128 k + 128 v
    nkt = hid // 128 + 1           # projection k-tiles, incl. the bias/ones pad tile
    n_ch = (t + 511) // 512        # 512-wide score chunks over past tokens
    n_tc = t // 128                # 128-wide probsT chunks
    n_pm = (b + 3) // 4            # probs holder tiles (4 batches each)
    n_pr = b // 2                  # batch pairs
    WCH = 4                        # wT k-tiles per DMA chunk

    nc = bacc.Bacc("TRN2")
    idn = nc.dram_tensor("ident", [128, 128], f32, kind="ExternalInput")
    xT = nc.dram_tensor("xT", [128, nkt, b], f32r, kind="ExternalInput")
    wT = nc.dram_tensor("wT", [128, nkt, od], f32r, kind="ExternalInput")
    kT = nc.dram_tensor("kT", [n_pr, D, 2, t], f32r, kind="ExternalInput")
    vv = nc.dram_tensor("v", [n_pr, 128, n_tc, 2 * D], f32r, kind="ExternalInput")
    mk = nc.dram_tensor("mask", [b, t + 1], f32, kind="ExternalInput")
    o_av = nc.dram_tensor("av", [n_pr, 2 * GQ, 2 * D], f32, kind="ExternalOutput")
    o_sum = nc.dram_tensor("sums", [n_pm, 128, n_ch], f32, kind="ExternalOutput")
    o_sn = nc.dram_tensor("snew", [GQ * b, b], f32, kind="ExternalOutput")
    o_kv = nc.dram_tensor("kv_new", [b, 2 * D], f32, kind="ExternalOutput")

    dmae = [nc.sync, nc.scalar, nc.gpsimd]

    with tile.TileContext(nc) as tc:
        with (
            tc.tile_pool(name="singles", bufs=1) as singles,
            tc.tile_pool(name="ktp", bufs=2) as ktp,
            tc.tile_pool(name="vp", bufs=3) as vp,
            tc.tile_pool(name="ptp", bufs=2) as ptp,
            tc.tile_pool(name="mrow", bufs=2) as mrowp,
            tc.tile_pool(name="small", bufs=4) as small,
        ):
            ident = singles.tile([128, 128], f32)
            nc.scalar.dma_start(ident, idn[:, :])
            if with_mask:
                ones1 = singles.tile([1, GQ], f32)
                nc.vector.memset(ones1, 1.0)

            # ---- P1: fused qkv projection: qkv[b, od] = xT.T @ wT ----
            with (
                tc.tile_pool(name="proj", bufs=3) as proj,
                tc.tile_pool(name="qkvp", bufs=1, space="PSUM") as qkvp,
            ):
                xt_sb = singles.tile([128, nkt, b], f32r)
                nc.scalar.dma_start(xt_sb, xT[:, :, :])
                qkv_ps = qkvp.tile([b, od], f32)
                for c in range((nkt + WCH - 1) // WCH):
                    i0 = c * WCH
                    ii = min(WCH, nkt - i0)
                    wt_c = proj.tile([128, WCH, od], f32r, tag="wt")
                    dmae[c % 3].dma_start(wt_c[:, :ii, :], wT[:, i0:i0 + ii, :])
                    for i in range(i0, i0 + ii):
                        for c0 in range(0, od, 512):
                            cc = min(512, od - c0)
                            nc.tensor.matmul(
                                qkv_ps[:, c0:c0 + cc], xt_sb[:, i, :],
                                wt_c[:, i - i0, c0:c0 + cc],
                                start=(i == 0), stop=(i == nkt - 1),
                            )
                qkv_sb = singles.tile([b, od], f32)
                nc.scalar.copy(qkv_sb, qkv_ps)
            nc.scalar.dma_start(o_kv[:, :], qkv_sb[:, GQ * D:])

            # ---- P2: transpose q heads and k_new to [d, *]; batched s_new ----
            qT_pack = singles.tile([128, GQ * b], f32r)
            k_newT = singles.tile([128, b], f32r)
            with tc.tile_pool(name="tp1", bufs=2, space="PSUM") as tp1:
                for h in range(GQ):
                    tp_ps = tp1.tile([128, b], f32, tag="tp")
                    nc.tensor.transpose(tp_ps, qkv_sb[:, h * D:(h + 1) * D], ident[:b, :b])
                    nc.vector.tensor_copy(qT_pack[:, h * b:(h + 1) * b], tp_ps)
                tp_ps = tp1.tile([128, b], f32, tag="tp")
                nc.tensor.transpose(tp_ps, qkv_sb[:, GQ * D:GQ * D + D], ident[:b, :b])
                nc.vector.tensor_copy(k_newT, tp_ps)
                # s_new[(h,b), b'] = q[(h,b)]·k_new[b']; host takes b'==b diagonal
                sn_ps = tp1.tile([GQ * b, b], f32, tag="sn")
                nc.tensor.matmul(sn_ps, qT_pack, k_newT, start=True, stop=True)
                sn_sb = singles.tile([GQ * b, b], f32)
                nc.vector.tensor_copy(sn_sb, sn_ps)
                nc.scalar.dma_start(o_sn[:, :], sn_sb)

            qT_r = qT_pack.rearrange("d (h b) -> d b h", b=b)

            # probs holders: 4 batches per [128, t] tile at 32-aligned bases
            probs = [singles.tile([128, t], f32, name=f"probs{m}", tag=f"probs{m}")
                     for m in range(n_pm)]
            sums = [singles.tile([128, n_ch], f32, name=f"sums{m}", tag=f"sums{m}")
                    for m in range(n_pm)]
            for m in range(n_pm):
                nc.vector.memset(sums[m], 0.0)

            # ---- main per-pair pipeline: scores -> exp -> probsT -> AV ----
            with (
                tc.tile_pool(name="scp", bufs=1, space="PSUM") as scp,
                tc.tile_pool(name="ptpp", bufs=1, space="PSUM") as ptpp,
                tc.tile_pool(name="avp", bufs=2, space="PSUM") as avp,
            ):
                for p in range(n_pr):
                    kt_sb = ktp.tile([D, 2, t], f32r, tag="kt")
                    nc.sync.dma_start(kt_sb, kT[p, :, :, :])
                    v_sb = vp.tile([128, n_tc, 2 * D], f32r, tag="v")
                    nc.gpsimd.dma_start(v_sb, vv[p, :, :, :])
                    tpp = ptpp.tile([128, 8 * n_tc], f32, tag="tpp")

                    for j in (0, 1):
                        bi = 2 * p + j
                        m_i, g_i = bi // 4, 32 * (bi % 4)
                        lhq = qT_r[:, bi, :]
                        if with_mask:
                            mrow = mrowp.tile([1, t + 1], f32, tag="m")
                            nc.gpsimd.dma_start(mrow, mk[bi:bi + 1, :])
                        for c in range(n_ch):
                            c0 = 512 * c
                            cc = min(512, t - c0)
                            sc_c = scp.tile([GQ, 512], f32, name=f"sc{c}", tag=f"sc{c}")
                            nc.tensor.matmul(sc_c[:, :cc], lhq,
                                             kt_sb[:, j, c0:c0 + cc],
                                             start=True, stop=not with_mask)
                            if with_mask:
                                nc.tensor.matmul(sc_c[:, :cc], ones1,
                                                 mrow[:, c0:c0 + cc],
                                                 start=False, stop=True)
                            nc.scalar.activation(
                                out=probs[m_i][g_i:g_i + GQ, c0:c0 + cc],
                                in_=sc_c[:, :cc],
                                func=mybir.ActivationFunctionType.Exp,
                                scale=float(SCALE),
                                accum_out=sums[m_i][g_i:g_i + GQ, c:c + 1],
                            )
                        for i in range(n_tc):
                            nc.tensor.transpose(
                                tpp[:, 8 * i + 4 * j:8 * i + 4 * j + 4],
                                probs[m_i][g_i:g_i + GQ, 128 * i:128 * (i + 1)],
                                ident[g_i:g_i + GQ, g_i:g_i + GQ],
                                tile_position=(g_i, 0),
                            )

                    pt_sb = ptp.tile([128, 8 * n_tc], f32r, tag="pt")
                    nc.vector.tensor_copy(pt_sb, tpp)
                    av_ps = avp.tile([2 * GQ, 2 * D], f32, tag="av")
                    for i in range(n_tc):
                        nc.tensor.matmul(
                            av_ps, pt_sb[:, 8 * i:8 * i + 8], v_sb[:, i, :],
                            start=(i == 0), stop=(i == n_tc - 1),
                        )
                    av_sb = small.tile([2 * GQ, 2 * D], f32, tag="av_sb")
                    nc.vector.tensor_copy(av_sb, av_ps)
                    nc.sync.dma_start(o_av[p, :, :], av_sb)

            for m in range(n_pm):
                nc.scalar.dma_start(o_sum[m, :, :], sums[m][:, :])

    nc.compile()
    return nc


def _get_nc(b, t, hid, with_mask):
    key = (b, t, hid, with_mask)
    if key not in _CACHE:
        _CACHE[key] = build_nc(b, t, hid, with_mask)
    return _CACHE[key]


def _tile_k(a, pad_rows):
    """[K, N] -> [128, K//128, N] with zero padding to pad_rows."""
    k, n = a.shape
    out = np.zeros((pad_rows, n), np.float32)
    out[:k] = a
    return np.ascontiguousarray(out.reshape(pad_rows // 128, 128, n).transpose(1, 0, 2))


def make_core_inputs(x, W, bias, key_cache, value_cache, mask, core, b, t, hid):
    """Build the per-core input map (host-side layout only)."""
    gqd = GQ * D
    nkt = hid // 128 + 1
    xe = np.concatenate([x.T, np.ones((1, b), np.float32)], 0)       # [hid+1, b]
    rows = np.concatenate([
        W[gqd * core:gqd * (core + 1)],
        W[NKV * gqd + D * core: NKV * gqd + D * (core + 1)],
        W[NKV * gqd + NKV * D + D * core: NKV * gqd + NKV * D + D * (core + 1)],
    ], 0)
    brows = np.concatenate([
        bias[gqd * core:gqd * (core + 1)],
        bias[NKV * gqd + D * core: NKV * gqd + D * (core + 1)],
        bias[NKV * gqd + NKV * D + D * core: NKV * gqd + NKV * D + D * (core + 1)],
    ], 0)
    we = np.concatenate([rows.T, brows[None, :]], 0)                 # [hid+1, od]
    kc = key_cache[:, :, core, :]                                    # [b, t, D]
    vc = value_cache[:, :, core, :]
    kTh = np.ascontiguousarray(kc.reshape(b // 2, 2, t, D).transpose(0, 3, 1, 2))
    vh = np.ascontiguousarray(
        vc.reshape(b // 2, 2, t // 128, 128, D).transpose(0, 3, 2, 1, 4)
        .reshape(b // 2, 128, t // 128, 2 * D))
    return {"xT": _tile_k(xe, nkt * 128), "wT": _tile_k(we, nkt * 128),
            "kT": kTh, "v": vh, "mask": mask,
            "ident": np.eye(128, dtype=np.float32)}


def kernel(input_t, key_cache, value_cache, max_position, attention_mask, beam_idx,
           W_qkv, b_qkv):
    from concourse.bass_utils import run_bass_kernel_spmd

    b, t, hid = B, PAST, HID
    n_ch = (t + 511) // 512
    x = np.asarray(input_t, np.float32).reshape(b, hid)
    W = np.asarray(W_qkv, np.float32)
    bias = np.asarray(b_qkv, np.float32)
    kc = np.asarray(key_cache, np.float32)
    vc = np.asarray(value_cache, np.float32)
    mask_raw = np.asarray(attention_mask, np.float32).reshape(b, t + 1)
    # pre-scale the additive mask so exp(scale*(s + mask')) == exp(scale*s + mask)
    mask = np.ascontiguousarray(mask_raw / SCALE)
    with_mask = bool(np.any(mask_raw))

    nc = _get_nc(b, t, hid, with_mask)
    in_maps = [make_core_inputs(x, W, bias, kc, vc, mask, c, b, t, hid)
               for c in range(NKV)]
    global LAST_RESULT
    LAST_RESULT = run_bass_kernel_spmd(nc, in_maps, list(range(NKV)), trace=TRACE)
    res = LAST_RESULT.results

    out = np.empty((b, NH, 1, D), np.float32)
    k_new = np.empty((b, 1, NKV, D), np.float32)
    v_new = np.empty((b, 1, NKV, D), np.float32)
    n_pm = (b + 3) // 4
    bidx = np.arange(b)
    for c in range(NKV):
        r = res[c]
        kv = r["kv_new"]                              # [b, 256]
        k_new[:, 0, c, :] = kv[:, :D]
        v_new[:, 0, c, :] = kv[:, D:]
        # av pair blocks: [b/2, 2*GQ, 2*D]; batch 2p+j at rows 4j:4j+4, cols Dj:+D
        a2 = r["av"].reshape(b // 2, 2 * GQ, 2 * D)
        av = np.empty((b, GQ, D), np.float32)
        av[0::2] = a2[:, 0:GQ, 0:D]
        av[1::2] = a2[:, GQ:2 * GQ, D:2 * D]
        # sums packed [n_pm, 128, n_ch]; batch 4m+g at [m, 32g:32g+GQ, :]
        sm = r["sums"].reshape(n_pm, 4, 32, n_ch)[:, :, :GQ, :].sum(-1).reshape(b, GQ)
        # s_new diag: snew[(h,b), b'] -> [h*b? rows are h*B + bi, take col bi]
        sn = r["snew"].reshape(GQ, b, b)[:, bidx, bidx].T     # [b, GQ]
        pn = np.exp(SCALE * sn + mask_raw[:, t:t + 1])
        o = (av + pn[:, :, None] * kv[:, None, D:]) / (sm + pn)[:, :, None]
        out[:, GQ * c:GQ * (c + 1), 0, :] = o
    k_cat = np.concatenate([kc, k_new], axis=1)
    v_cat = np.concatenate([vc, v_new], axis=1)
    return out, k_cat, v_cat
